# revision 14
# baseline (speedup 1.0000x reference)
"""LIF (leaky integrate-and-fire) forward kernel for Trainium2, 8 NeuronCores.

Recurrence (per element of [B, N], serial over T):
    v_t = DECAY * (v_{t-1} * (1 - s_{t-1})) + x_t      (REST = 0)
    s_t = (v_t > THRESHOLD)

Lanes (columns of the per-core [128, 2048] slab):
  D-lane  cols [0, C_D):        exact fp32 recurrence on DVE (2 fused STTs)
  P1/P2 sub-lanes (C_D..2048):  TensorE computes v in PSUM:
      psv = (DECAY I)^T w  [fp32r, 1cyc/row]  + I^T x_hi + I^T x_lo  [bf16]
      m   = Sign(psv - THR)   ScalarE -> bf16
      w   = (m is_le 0)*psv   DVE STT -> fp32r state
  Two sub-lanes pipeline the PE -> ScalarE -> DVE -> PE chain across steps.

Output: TensorE packs 8 steps of m into a PSUM byte-plane
(psum += 2^k I^T m_k, emitted one step late to fill PE stalls), ScalarE
drains it as int8 (B-128), 1 byte per 8 steps per element.

Input feeds: cols [0,C_D) fp32 plane; cols [C_D,2048) bf16 hi+lo planes
(same bytes/elem as fp32). All per-core tensors are p-major so chunked
DMA loads are contiguous 16-32 KiB per partition.
"""

import numpy as np
import ml_dtypes

import concourse.bacc as bacc
import concourse.mybir as mybir
from concourse.tile import TileContext
from concourse.bass_utils import run_bass_kernel_spmd

T, B, N = 32, 128, 16384
N_CORES = 8
B_SH = B // N_CORES          # 16 batch rows per core
S = B_SH * N                 # 262144 elements per core per time step
P = 128                      # SBUF partitions
F = S // P                   # 2048 free-dim elements
DECAY = 0.2
THR = 0.3

C_D = 256                    # columns on the exact DVE lane
C_G = 256                    # columns on the GpSimd arith lane
C_DG = C_D + C_G             # fp32-fed columns (the "xd" plane)
C_P = F - C_DG               # columns on the TensorE lane
SUB = (1024, 512)            # P-lane sub-lane widths (sum == C_P)
CHUNK = 4                    # time steps per input DMA
GROUPS = T // 8              # byte-planes (8 steps packed per byte)

TRACE = False                # set True (e.g. from test.py) to capture a profile

_BUILT = {}


def _build_nc():
    nc = bacc.Bacc("TRN2", debug=False, num_devices=N_CORES)
    f32 = mybir.dt.float32
    f32r = mybir.dt.float32r
    bf16 = mybir.dt.bfloat16
    Alu = mybir.AluOpType
    Act = mybir.ActivationFunctionType

    xd = nc.dram_tensor("xd", [P, T * C_DG], f32, kind="ExternalInput").ap()
    xh = nc.dram_tensor("xh", [P, T * C_P], bf16, kind="ExternalInput").ap()
    xl = nc.dram_tensor("xl", [P, T * C_P], bf16, kind="ExternalInput").ap()
    wp_in = nc.dram_tensor("wpack", [P, 8 * P], bf16, kind="ExternalInput").ap()
    dec_in = nc.dram_tensor("decayi", [P, P], f32, kind="ExternalInput").ap()
    ib_in = nc.dram_tensor("identb", [P, P], bf16, kind="ExternalInput").ap()
    y = nc.dram_tensor("y", [P, GROUPS * F], mybir.dt.int8,
                       kind="ExternalOutput").ap()
    xdr = xd.rearrange("p (t f) -> p t f", t=T)
    xhr = xh.rearrange("p (t f) -> p t f", t=T)
    xlr = xl.rearrange("p (t f) -> p t f", t=T)
    yr = y.rearrange("p (g f) -> p g f", g=GROUPS)

    sub_off = [0]
    for wdt in SUB:
        sub_off.append(sub_off[-1] + wdt)

    with TileContext(nc) as tc:
        with (
            tc.tile_pool(name="consts", bufs=1) as c_pool,
            tc.tile_pool(name="state", bufs=1) as state_pool,
            tc.tile_pool(name="xin", bufs=3) as xin_pool,
            tc.tile_pool(name="vtmp", bufs=2) as v_pool,
            tc.tile_pool(name="mtile", bufs=3) as m_pool,
            tc.tile_pool(name="outs", bufs=2) as o_pool,
            tc.tile_pool(name="psv1", bufs=1, space="PSUM") as pv1_pool,
            tc.tile_pool(name="psv2", bufs=1, space="PSUM") as pv2_pool,
            tc.tile_pool(name="pack", bufs=1, space="PSUM") as pk_pool,
        ):
            negthr = nc.alloc_sbuf_tensor("const_negthr", [P, 1], f32).ap()
            nc.gpsimd.memset(negthr, -THR)
            neghalf = nc.alloc_sbuf_tensor("const_neghalf", [P, 1], f32).ap()
            nc.gpsimd.memset(neghalf, -0.5)

            wsb = c_pool.tile([P, 8 * P], bf16)
            nc.sync.dma_start(out=wsb[:], in_=wp_in)
            decf = c_pool.tile([P, P], f32)
            nc.sync.dma_start(out=decf[:], in_=dec_in)
            identb = c_pool.tile([P, P], bf16)
            nc.sync.dma_start(out=identb[:], in_=ib_in)
            decr = c_pool.tile([P, P], f32r)
            nc.vector.scalar_tensor_tensor(
                out=decr[:], in0=decf[:], scalar=1.0, in1=decf[:],
                op0=Alu.bypass, op1=Alu.bypass,
            )

            wd = state_pool.tile([P, C_D], f32)       # D-lane state
            wg = state_pool.tile([P, C_G], f32)       # G-lane state (0.2*w)
            msg = state_pool.tile([P, C_G], f32)      # G-lane mask {0, 0.2}
            wp = state_pool.tile([P, C_P], f32r)      # P-lane state (rounded)

            pv_pools = (pv1_pool, pv2_pool)

            def emit_pack(m_prev, t_prev):
                k = t_prev % 8
                wk = wsb[:, k * P:(k + 1) * P]
                for r in range(0, F, 512):
                    nc.tensor.matmul(
                        out=pack_psum[:, r:r + 512], lhsT=wk,
                        rhs=m_prev[:, r:r + 512],
                        start=(k == 0), stop=(k == 7),
                    )

            def emit_drain(t_prev):
                g8 = t_prev // 8
                oi = o_pool.tile([P, F], mybir.dt.int8, name="oi")
                nc.scalar.activation(
                    oi[:], pack_psum[:], Act.Identity, bias=neghalf, scale=0.5)
                nc.scalar.dma_start(out=yr[:, g8, :], in_=oi[:])

            xdt = xht = xlt = None
            m_prev = None
            pack_psum = None
            for t in range(T):
                j = t % CHUNK
                if j == 0:
                    xdt = xin_pool.tile([P, CHUNK * C_DG], f32, name="xdt")
                    xht = xin_pool.tile([P, CHUNK * C_P], bf16, name="xht")
                    xlt = xin_pool.tile([P, CHUNK * C_P], bf16, name="xlt")
                    if t == 0:
                        for jj in range(CHUNK):
                            nc.sync.dma_start(
                                out=xht[:, jj * C_P:(jj + 1) * C_P],
                                in_=xhr[:, jj, :])
                            nc.sync.dma_start(
                                out=xlt[:, jj * C_P:(jj + 1) * C_P],
                                in_=xlr[:, jj, :])
                            nc.sync.dma_start(
                                out=xdt[:, jj * C_DG:(jj + 1) * C_DG],
                                in_=xdr[:, jj, :])
                    else:
                        nc.sync.dma_start(out=xht[:], in_=xhr[:, t:t + CHUNK, :])
                        nc.sync.dma_start(out=xlt[:], in_=xlr[:, t:t + CHUNK, :])
                        nc.sync.dma_start(out=xdt[:], in_=xdr[:, t:t + CHUNK, :])
                xds = xdt[:, j * C_DG:(j + 1) * C_DG]
                xgs = xds[:, C_D:]
                xhs = xht[:, j * C_P:(j + 1) * C_P]
                xls = xlt[:, j * C_P:(j + 1) * C_P]

                if t % 8 == 0:
                    pack_psum = pk_pool.tile([P, F], f32, name="pk")

                v = v_pool.tile([P, C_DG], f32, name="vt")
                m = m_pool.tile([P, F], bf16, name="mt")
                psvs = [
                    pool.tile([P, wdt], f32, name=f"ps{i}")
                    for i, (pool, wdt) in enumerate(zip(pv_pools, SUB))
                ]

                # deferred pack of the previous step fills PE stall time
                if m_prev is not None:
                    emit_pack(m_prev, t - 1)

                # --- P sub-lanes: v-matmuls + Sign, interleaved ---
                for i, wdt in enumerate(SUB):
                    o0 = sub_off[i]
                    psv = psvs[i]
                    # x-matmuls first (only DMA-dependent) so PE starts
                    # before the previous step's reset lands; decr last.
                    for g in range(0, wdt, 512):
                        a, bnd = o0 + g, o0 + min(g + 512, wdt)
                        nc.tensor.matmul(
                            out=psv[:, g:bnd - o0], lhsT=identb[:],
                            rhs=xhs[:, a:bnd], start=True, stop=False)
                    for g in range(0, wdt, 512):
                        a, bnd = o0 + g, o0 + min(g + 512, wdt)
                        nc.tensor.matmul(
                            out=psv[:, g:bnd - o0], lhsT=identb[:],
                            rhs=xls[:, a:bnd], start=False, stop=(t == 0))
                    if t > 0:
                        for g in range(0, wdt, 512):
                            a, bnd = o0 + g, o0 + min(g + 512, wdt)
                            nc.tensor.matmul(
                                out=psv[:, g:bnd - o0], lhsT=decr[:],
                                rhs=wp[:, a:bnd], start=False, stop=True)
                    nc.scalar.activation(
                        m[:, C_DG + o0:C_DG + o0 + wdt], psv[:],
                        Act.Sign, bias=negthr)
                    if i == 0:
                        # D-lane DVE work runs while ACT handles P1's Sign
                        if t == 0:
                            nc.vector.scalar_tensor_tensor(
                                out=wd[:], in0=xds[:, :C_D], scalar=THR,
                                in1=xds[:, :C_D], op0=Alu.is_le, op1=Alu.mult)
                        else:
                            nc.vector.scalar_tensor_tensor(
                                out=v[:, :C_D], in0=wd[:], scalar=DECAY,
                                in1=xds[:, :C_D], op0=Alu.mult, op1=Alu.add)
                            nc.vector.scalar_tensor_tensor(
                                out=wd[:], in0=v[:, :C_D], scalar=THR,
                                in1=v[:, :C_D], op0=Alu.is_le, op1=Alu.mult)
                        # G-lane: GpSimd arith + DVE fast mask
                        if t == 0:
                            vg = xgs
                        else:
                            nc.gpsimd.tensor_tensor(
                                out=v[:, C_D:], in0=wg[:], in1=xgs,
                                op=Alu.add)
                            vg = v[:, C_D:]
                        nc.vector.tensor_scalar(
                            out=msg[:], in0=vg, scalar1=THR, scalar2=DECAY,
                            op0=Alu.is_le, op1=Alu.mult)
                        nc.gpsimd.tensor_tensor(
                            out=wg[:], in0=msg[:], in1=vg, op=Alu.mult)

                # P sub-lane resets on DVE
                for i, wdt in enumerate(SUB):
                    o0 = sub_off[i]
                    nc.vector.scalar_tensor_tensor(
                        out=wp[:, o0:o0 + wdt], in0=m[:, C_DG + o0:C_DG + o0 + wdt],
                        scalar=0.0, in1=psvs[i][:], op0=Alu.is_le, op1=Alu.mult)

                # D+G lane Sign (ACT queue: after P sub-lane Signs)
                nc.scalar.activation(
                    m[:, :C_DG], xds if t == 0 else v[:], Act.Sign, bias=negthr)

                # group drain (once per 8 steps), ordered last on ScalarE
                if t > 0 and (t - 1) % 8 == 7:
                    emit_drain(t - 1)

                m_prev = m

            emit_pack(m_prev, T - 1)
            emit_drain(T - 1)
    nc.compile()
    return nc


LAST_RESULTS = None


def _make_consts():
    wp = np.zeros((P, 8 * P), dtype=np.float32)
    for k in range(8):
        wp[:, k * P:(k + 1) * P][np.arange(P), np.arange(P)] = float(2 ** k)
    wpack = (wp.view(np.uint32) >> 16).astype(np.uint16)
    dec = np.zeros((P, P), dtype=np.float32)
    dec[np.arange(P), np.arange(P)] = np.float32(DECAY)
    ib = np.zeros((P, P), dtype=ml_dtypes.bfloat16)
    ib[np.arange(P), np.arange(P)] = 1.0
    return wpack, dec, ib.view(np.uint16)


def kernel(tx):
    global LAST_RESULTS
    tx = np.asarray(tx)
    assert tx.shape == (T, B, N) and tx.dtype == np.float32

    if "nc" not in _BUILT:
        _BUILT["nc"] = _build_nc()
    nc = _BUILT["nc"]

    wpack, dec, ib = _make_consts()
    in_maps = []
    for c in range(N_CORES):
        xc = tx[:, c * B_SH:(c + 1) * B_SH, :].reshape(T, P, F)
        xc = np.ascontiguousarray(xc.transpose(1, 0, 2))     # [P, T, F]
        xdc = np.ascontiguousarray(xc[:, :, :C_DG]).reshape(P, T * C_DG)
        xpc = xc[:, :, C_DG:]
        xhc = xpc.astype(ml_dtypes.bfloat16)
        xlc = (xpc - xhc.astype(np.float32)).astype(ml_dtypes.bfloat16)
        in_maps.append({
            "xd": xdc,
            "xh": np.ascontiguousarray(xhc).reshape(P, T * C_P).view(np.uint16),
            "xl": np.ascontiguousarray(xlc).reshape(P, T * C_P).view(np.uint16),
            "wpack": wpack, "decayi": dec, "identb": ib,
        })

    res = run_bass_kernel_spmd(nc, in_maps, core_ids=list(range(N_CORES)), trace=TRACE)
    LAST_RESULTS = res

    out = np.empty((T, B, N), dtype=np.float32)
    for c in range(N_CORES):
        yb = np.asarray(res.results[c]["y"]).astype(np.int16)  # [P, GROUPS*F]
        Bv = (yb + 128).astype(np.uint8).reshape(P, GROUPS, F)
        for g in range(GROUPS):
            for k in range(8):
                bits = (Bv[:, g, :] >> k) & 1          # [P, F]
                st = bits.reshape(B_SH, N).astype(np.float32)
                out[g * 8 + k, c * B_SH:(c + 1) * B_SH, :] = st
    return out
